# revision 6
# baseline (speedup 1.0000x reference)
"""Nystrom attention (B=2, N=16384, C=512, H=8, hd=64, m=64 landmarks) on 8
Trainium2 NeuronCores.

Sharding: batch*heads across cores. Core c handles batch b = c//4 and heads
{2*(c%4), 2*(c%4)+1}. Each core reads x[b] (transposed) and produces its
2-head contribution to out[b] @ wo.T; the host sums the 4 partial
contributions per batch and adds bo.

The 64x64 ker2 = softmax(q_land @ k_land^T) and its SVD pseudo-inverse
(jnp.linalg.pinv semantics: drop singular values <= 10*64*eps*s_max) are
computed on the host: the final output is extremely sensitive to the exact
truncation cutoff, and SVD is not available on-device. Everything O(N) runs
on-device in fp32:

  qT/kT  [128, N]  = (wq*scale | wk)-rows^T.T @ x^T             (both heads)
  e1T    [64, N]   = exp(k_landT.T @ qT_h)                      (per head)
  e3T    [128n,64] = exp(kT_h-chunk.T @ q_landT)                (per head)
  kv,s3  [64, 65]  = accum_n e3T.T @ [v0 | 1 | v1]-slice        (per head)
  w2T    [64, 64]  = kv_norm-lhsT matmul with ker2_inv^T        (per head)
  W3     [64, 512] = w2T-lhsT matmul with wo_head^T             (per head)
  out    [N, 512]  = sum_h (e1T_h.T @ W3_h) / (e1T_h.T @ 1)

Landmarks are computed host-side from segment means of x (linear maps
commute with the segment mean; verified numerically equivalent).
"""

import numpy as np

import concourse.bass as bass
import concourse.tile as tile
from concourse import bacc, mybir
from concourse.bass_utils import run_bass_kernel_spmd

F32 = mybir.dt.float32
AF = mybir.ActivationFunctionType

B, N, C = 2, 16384, 512
H, HD, M = 8, 64, 64
SCALE = np.float32(HD ** -0.5)
N_CORES = 8
CHUNK = 512  # pass-1 n-chunk (max fp32 moving free dim)


def build_nc(n=N):
    nchunk = n // CHUNK
    ntile = n // 128

    nc = bacc.Bacc("TRN2", target_bir_lowering=False, debug=False)

    xT = nc.dram_tensor("xT", [C, n], F32, kind="ExternalInput")
    wqT = nc.dram_tensor("wqT", [C, 128], F32, kind="ExternalInput")
    wkT = nc.dram_tensor("wkT", [C, 128], F32, kind="ExternalInput")
    wvT = nc.dram_tensor("wvT", [C, 128], F32, kind="ExternalInput")
    klT = nc.dram_tensor("klT", [128, M], F32, kind="ExternalInput")   # [h*64+d, m]
    qlT = nc.dram_tensor("qlT", [128, M], F32, kind="ExternalInput")   # [h*64+d, m]
    k2iT = nc.dram_tensor("k2iT", [M, 2, M], F32, kind="ExternalInput")  # [m, h, m']
    woT = nc.dram_tensor("woT", [HD, 2, C], F32, kind="ExternalInput")   # [d, h, c]
    out_p = nc.dram_tensor("out_p", [n, C], F32, kind="ExternalOutput")

    with tile.TileContext(nc) as tc, tc.tile_pool(name="const", bufs=1) as const:
        w_q = const.tile([128, 4, 128], F32, tag="w_q")
        w_k = const.tile([128, 4, 128], F32, tag="w_k")
        w_v = const.tile([128, 4, 128], F32, tag="w_v")
        nc.sync.dma_start(w_q[:], wqT.rearrange("(a p) d -> p a d", p=128))
        nc.sync.dma_start(w_k[:], wkT.rearrange("(a p) d -> p a d", p=128))
        nc.sync.dma_start(w_v[:], wvT.rearrange("(a p) d -> p a d", p=128))
        kl = const.tile([128, M], F32, tag="kl")
        ql = const.tile([128, M], F32, tag="ql")
        k2i = const.tile([M, 2, M], F32, tag="k2i")
        wo_t = const.tile([HD, 2, C], F32, tag="wo_t")
        nc.sync.dma_start(kl[:], klT[:])
        nc.sync.dma_start(ql[:], qlT[:])
        nc.sync.dma_start(k2i[:], k2iT[:])
        nc.sync.dma_start(wo_t[:], woT[:])
        ones64 = const.tile([M, 1], F32, tag="ones64")
        nc.vector.memset(ones64[:], 1.0)

        e1t = [const.tile([M, n], F32, tag=f"e1t{h}", name=f"e1t{h}")
               for h in range(2)]
        w3s = const.tile([M, 2, C], F32, tag="w3s")

        # kv accumulators persist across pass 1 + the between-pass stage,
        # then their psum banks are released before pass 2.
        with tc.tile_pool(name="kvps", bufs=1, space="PSUM") as kvps:
            kvp = [kvps.tile([M, M + 1], F32, tag=f"kvp{h}", name=f"kvp{h}")
                   for h in range(2)]

            # ---------------- pass 1 ----------------
            with (
                tc.tile_pool(name="xc", bufs=3) as xcp,
                tc.tile_pool(name="projps", bufs=2, space="PSUM") as projps,
                tc.tile_pool(name="logps", bufs=2, space="PSUM") as logps,
                tc.tile_pool(name="vps", bufs=2, space="PSUM") as vps,
                tc.tile_pool(name="projsb", bufs=3) as projsb,
                tc.tile_pool(name="vsb", bufs=3) as vsb,
                tc.tile_pool(name="e3sb", bufs=4) as e3sb,
            ):
                for ci in range(nchunk):
                    n0 = ci * CHUNK
                    xc = xcp.tile([128, 4, CHUNK], F32, tag="xc")
                    nc.sync.dma_start(
                        xc[:], xT[:, n0:n0 + CHUNK].rearrange("(a p) n -> p a n", p=128)
                    )

                    qp = projps.tile([128, CHUNK], F32, tag="proj")
                    for a in range(4):
                        nc.tensor.matmul(qp[:], w_q[:, a, :], xc[:, a, :],
                                         start=(a == 0), stop=(a == 3))
                    qs = projsb.tile([128, CHUNK], F32, tag="qs")
                    nc.scalar.copy(qs[:], qp[:])

                    kp = projps.tile([128, CHUNK], F32, tag="proj")
                    for a in range(4):
                        nc.tensor.matmul(kp[:], w_k[:, a, :], xc[:, a, :],
                                         start=(a == 0), stop=(a == 3))
                    ks = projsb.tile([128, CHUNK], F32, tag="ks")
                    nc.scalar.copy(ks[:], kp[:])

                    for h in range(2):
                        hs = slice(h * HD, (h + 1) * HD)
                        l1p = logps.tile([M, CHUNK], F32, tag="log")
                        nc.tensor.matmul(l1p[:], kl[hs, :], qs[hs, :],
                                         start=True, stop=True)
                        nc.scalar.activation(e1t[h][:, n0:n0 + CHUNK], l1p[:], AF.Exp)

                    for sub in range(4):
                        s0 = sub * 128
                        vp = vps.tile([128, 128], F32, tag="vp")
                        for a in range(4):
                            nc.tensor.matmul(vp[:], xc[:, a, s0:s0 + 128], w_v[:, a, :],
                                             start=(a == 0), stop=(a == 3))
                        vs = vsb.tile([128, 2 * HD + 1], F32, tag="vs")
                        nc.scalar.copy(vs[:, 0:HD], vp[:, 0:HD])
                        nc.vector.memset(vs[:, HD:HD + 1], 1.0)
                        nc.scalar.copy(vs[:, HD + 1:2 * HD + 1], vp[:, HD:2 * HD])

                        first = (ci == 0 and sub == 0)
                        last = (ci == nchunk - 1 and sub == 3)
                        for h in range(2):
                            hs = slice(h * HD, (h + 1) * HD)
                            l3p = logps.tile([128, M], F32, tag="log")
                            nc.tensor.matmul(l3p[:], ks[hs, s0:s0 + 128], ql[hs, :],
                                             start=True, stop=True)
                            e3s = e3sb.tile([128, M], F32, tag="e3s")
                            nc.scalar.activation(e3s[:], l3p[:], AF.Exp)
                            # rhs: head0 -> [v0 | 1], head1 -> [1 | v1]
                            nc.tensor.matmul(kvp[h][:], e3s[:],
                                             vs[:, h * HD:(h + 1) * HD + 1],
                                             start=first, stop=last)

            # ---------------- between passes ----------------
            with (
                tc.tile_pool(name="mid", bufs=2) as mid,
                tc.tile_pool(name="midps", bufs=2, space="PSUM") as midps,
            ):
                for h in range(2):
                    scol = HD if h == 0 else 0
                    vcol = 0 if h == 0 else 1
                    r3 = mid.tile([M, 1], F32, tag="r3")
                    nc.vector.reciprocal(r3[:], kvp[h][:, scol:scol + 1])
                    kvs = mid.tile([M, M], F32, tag="kvs")
                    nc.vector.tensor_scalar_mul(kvs[:], kvp[h][:, vcol:vcol + M], r3[:])
                    w2tp = midps.tile([M, M], F32, tag="w2tp")
                    nc.tensor.matmul(w2tp[:], kvs[:], k2i[:, h, :], start=True, stop=True)
                    w2ts = mid.tile([M, M], F32, tag="w2ts")
                    nc.scalar.copy(w2ts[:], w2tp[:])
                    w3p = midps.tile([M, C], F32, tag="w3p")
                    nc.tensor.matmul(w3p[:], w2ts[:], wo_t[:, h, :], start=True, stop=True)
                    nc.scalar.copy(w3s[:, h, :], w3p[:])

        # ---------------- pass 2 ----------------
        with (
            tc.tile_pool(name="ops", bufs=2, space="PSUM") as ops,
            tc.tile_pool(name="sps", bufs=2, space="PSUM") as sps,
            tc.tile_pool(name="osb", bufs=3) as osb,
            tc.tile_pool(name="rsb", bufs=4) as rsb,
        ):
            for ti in range(ntile):
                t0 = ti * 128
                oacc = None
                for h in range(2):
                    e1sl = e1t[h][:, t0:t0 + 128]
                    op = ops.tile([128, C], F32, tag="op")
                    nc.tensor.matmul(op[:], e1sl, w3s[:, h, :], start=True, stop=True)
                    sp = sps.tile([128, 1], F32, tag="sp")
                    nc.tensor.matmul(sp[:], e1sl, ones64[:], start=True, stop=True)
                    r1 = rsb.tile([128, 1], F32, tag="r1")
                    nc.vector.reciprocal(r1[:], sp[:])
                    ot = osb.tile([128, C], F32, tag=f"ot{h}")
                    nc.vector.tensor_scalar_mul(ot[:], op[:], r1[:])
                    if h == 0:
                        oacc = ot
                    else:
                        osum = osb.tile([128, C], F32, tag="osum")
                        nc.vector.tensor_add(osum[:], oacc[:], ot[:])
                        nc.sync.dma_start(out_p[t0:t0 + 128, :], osum[:])

    nc.compile()
    return nc


_NC_CACHE = {}


def _get_nc(n):
    if n not in _NC_CACHE:
        _NC_CACHE[n] = build_nc(n)
    return _NC_CACHE[n]


def _softmax(a):
    e = np.exp(a - a.max(-1, keepdims=True))
    return e / e.sum(-1, keepdims=True)


def _jax_pinv(a):
    """Match jnp.linalg.pinv float32 default semantics."""
    u, s, vh = np.linalg.svd(a.astype(np.float32))
    cutoff = 10.0 * max(a.shape[-2:]) * np.finfo(np.float32).eps * s[..., 0:1]
    keep = s > cutoff
    sinv = np.where(keep, 1.0 / np.where(keep, s, 1.0), 0.0).astype(np.float32)
    return np.einsum('...ji,...j,...kj->...ik', vh, sinv, u).astype(np.float32)


def _prep_in_maps(x, wq, bq, wk, bk, wv, bv, wo, bo):
    x = np.ascontiguousarray(np.asarray(x, dtype=np.float32))
    wq = np.asarray(wq, dtype=np.float32)
    wk = np.asarray(wk, dtype=np.float32)
    wv = np.asarray(wv, dtype=np.float32)
    wo = np.asarray(wo, dtype=np.float32)
    bq = np.asarray(bq, dtype=np.float32)
    bk = np.asarray(bk, dtype=np.float32)
    bv = np.asarray(bv, dtype=np.float32)
    assert not bq.any() and not bk.any() and not bv.any(), \
        "kernel specialized to zero q/k/v biases"

    b_, n, c = x.shape
    seg = n // M

    wq_s = wq * SCALE  # exact: SCALE is a power of two

    # ---- host: landmarks, ker2, pinv (O(N*C) + tiny) ----
    xm = x.reshape(b_, M, seg, c).mean(axis=2)                    # [B, m, C]
    q_land = (xm @ wq_s.T).reshape(b_, M, H, HD).transpose(0, 2, 1, 3)  # [B,H,m,hd]
    k_land = (xm @ wk.T).reshape(b_, M, H, HD).transpose(0, 2, 1, 3)
    ker2 = _softmax(np.einsum('bhld,bhmd->bhlm', q_land, k_land))
    k2inv = _jax_pinv(ker2)                                        # [B,H,m,m]

    xTb = [np.ascontiguousarray(x[b].T) for b in range(b_)]        # [C, N]
    in_maps = []
    for core in range(N_CORES):
        b = core // 4
        h0 = 2 * (core % 4)
        rows = slice(h0 * HD, (h0 + 2) * HD)   # 128 output dims of this head pair
        im = {
            "xT": xTb[b],
            "wqT": np.ascontiguousarray(wq_s[rows, :].T),
            "wkT": np.ascontiguousarray(wk[rows, :].T),
            "wvT": np.ascontiguousarray(wv[rows, :].T),
            # [h*64+d, m] stacks: head j of the pair at partitions j*64..j*64+63
            "klT": np.ascontiguousarray(
                np.concatenate([k_land[b, h0 + j].T for j in range(2)], axis=0)),
            "qlT": np.ascontiguousarray(
                np.concatenate([q_land[b, h0 + j].T for j in range(2)], axis=0)),
            "k2iT": np.ascontiguousarray(
                np.stack([k2inv[b, h0 + j].T for j in range(2)], axis=1)),
            "woT": np.ascontiguousarray(
                np.stack([wo[:, (h0 + j) * HD:(h0 + j + 1) * HD].T
                          for j in range(2)], axis=1)),
        }
        in_maps.append(im)
    return in_maps


def kernel(x, wq, bq, wk, bk, wv, bv, wo, bo):
    x = np.asarray(x, dtype=np.float32)
    bo = np.asarray(bo, dtype=np.float32)
    b_, n, c = x.shape

    in_maps = _prep_in_maps(x, wq, bq, wk, bk, wv, bv, wo, bo)
    nc = _get_nc(n)
    res = run_bass_kernel_spmd(nc, in_maps, core_ids=list(range(N_CORES)))

    out = np.empty((b_, n, c), dtype=np.float32)
    for b in range(b_):
        acc = res.results[4 * b + 0]["out_p"]
        for g in range(1, 4):
            acc = acc + res.results[4 * b + g]["out_p"]
        out[b] = acc + bo
    return out
